# revision 30
# baseline (speedup 1.0000x reference)
"""Trainium2 Bass kernel for nn_DWT_Layer: 3-level 2D db4 DWT (symmetric mode).

Input  x: (16, 3, 1024, 1024) fp32.
Output:   (16, 3, 64, 128, 128) fp32 — the flattened/truncated wavelet pyramid
          [cA3, cH3, cV3, cD3, cH2, cV2, cD2, cH1, cV1, cD1(truncated)].

Sharding: pure data parallel — 48 (batch*channel) images, 6 per core on 8
NeuronCores, no communication.

Algorithm (all-PE, transpose-free): each 1D DWT pass along the partition
axis is a set of banded fp16 matmuls out[m,j] = sum_r A[r,m]*M2[j,r] with
the DATA as lhsT and the folded/stacked DWT band matrix as rhs. The
contraction rows are stored in overlapping 128-row "slots" (stride <=122)
so that every output row j is owned by exactly one slot -> each psum
column is written by a single start=stop matmul (no accumulation, no
pre-zeroing) and the output comes out transposed. Running the same pass
twice (height then width) returns to row-major orientation, so the whole
3-level pyramid needs zero transposes, zero DVE MAC chains and zero
mirror ops: just cast-DMAs in, banded matmuls, PSUM->SBUF copies
(fp32->fp16 for the next stage / fp32 for detail staging) and row DMAs
out.
"""
import numpy as np

# ----------------------------------------------------------------- constants
DEC_LO = np.array([-0.010597401784997278, 0.032883011666982945,
                   0.030841381835986965, -0.18703481171888114,
                   -0.027983769416983849, 0.63088076792959036,
                   0.71484657055254153, 0.23037781330885523], dtype=np.float64)
L = 8
DEC_HI = np.array([(-1.0) ** (k + 1) * DEC_LO[L - 1 - k] for k in range(L)],
                  dtype=np.float64)

B, C, H, W = 16, 3, 1024, 1024
N_CORES = 8
IMGS_PER_CORE = 6
IMG_ELEMS = H * W

LEVEL_NS = [1024, 515, 261]   # input edge length per level


def nprime(N):
    return (N + 5) // 2 + 1


def ext_index(j, N):
    if j < 6:
        return 5 - j
    if j < N + 6:
        return j - 6
    return 2 * N + 5 - j


def dwt_matrix(N, filt):
    Np = nprime(N)
    M = np.zeros((Np, N), dtype=np.float64)
    filtrev = filt[::-1]
    for i in range(Np):
        for t in range(L):
            M[i, ext_index(2 * i + t, N)] += filtrev[t]
    return M


def build_slots(N):
    """[(o, j0, j1)]: slot covers input rows [o, o+128); owns outputs
    [j0, j1) whose (lo and hi) supports lie inside the slot."""
    Np = nprime(N)
    Mlo = dwt_matrix(N, DEC_LO)
    Mhi = dwt_matrix(N, DEC_HI)
    lo_r, hi_r = [], []
    for j in range(Np):
        nz = np.nonzero(np.abs(Mlo[j]) + np.abs(Mhi[j]))[0]
        lo_r.append(int(nz.min()))
        hi_r.append(int(nz.max()))
    slots = []
    j = 0
    while j < Np:
        o = min(max(0, lo_r[j]), N - 128)
        j1 = j
        while j1 < Np and lo_r[j1] >= o and hi_r[j1] < o + 128:
            j1 += 1
        assert j1 > j
        slots.append((o, j, j1))
        j = j1
    return slots


BW = 64  # max band width per slot


def build_weights(N):
    """fp16 packed rhs bands: [128, n_slots, 2, BW]."""
    slots = build_slots(N)
    Mlo = dwt_matrix(N, DEC_LO)
    Mhi = dwt_matrix(N, DEC_HI)
    arr = np.zeros((128, len(slots), 2, BW), dtype=np.float16)
    for s, (o, j0, j1) in enumerate(slots):
        assert j1 - j0 <= BW
        arr[:, s, 0, :j1 - j0] = Mlo[j0:j1, o:o + 128].T
        arr[:, s, 1, :j1 - j0] = Mhi[j0:j1, o:o + 128].T
    return arr


SLOTS = {N: build_slots(N) for N in LEVEL_NS}
WEIGHTS = {N: build_weights(N) for N in LEVEL_NS}

# p2 m-chunk offsets (within the 2Np free axis of B) per level:
#   lo chunks = next level's A slots (aa feeds them 1:1); hi chunks cover the
#   hi half with non-overlapping 128s + one 128-tail.
P2_LO = {1024: [o for o, _, _ in SLOTS[515]],   # {0,122,244,366,387}
         515: [o for o, _, _ in SLOTS[261]],    # {0,122,133}
         261: [0, 6]}
P2_HI = {1024: [515 + r for r in (0, 128, 256, 384, 387)],
         515: [261 + r for r in (0, 128, 133)],
         261: [134 + r for r in (0, 6)]}

# output section offsets (elements within one image's 1048576-long output)
SECT = {}
_cur = 0
for _name, _n in [("cA3", 134), ("cH3", 134), ("cV3", 134), ("cD3", 134),
                  ("cH2", 261), ("cV2", 261), ("cD2", 261),
                  ("cH1", 515), ("cV1", 515), ("cD1", 515)]:
    SECT[_name] = (_cur, _n)
    _cur += _n * _n
CD1_FULL_ROWS = 469
CD1_PART_COLS = 404
assert SECT["cD1"][0] + CD1_FULL_ROWS * 515 + CD1_PART_COLS == IMG_ELEMS

AB_BUFS = 2      # buffers for the A/B activation tiles
ST515_BUFS = 6   # fp32 detail-staging buffers (3 live per image)

_BUILT = None


def _split_512(a, b):
    """split [a, b) at multiples of 512 (psum bank boundaries)."""
    out = []
    while a < b:
        e = min(b, (a // 512 + 1) * 512)
        out.append((a, e))
        a = e
    return out


def _emit_pass(nc, psp, N, lhsT_of, wsb, sink, copy_engines):
    """One DWT pass: for each m-chunk, banded matmuls into two psum tiles
    (independent lo/hi pipelining lanes), then sink(ci, (lo, hi)) emits the
    copies out of psum."""
    Np = nprime(N)
    slots = SLOTS[N]
    chunks = sink.chunks
    f32 = _dt().float32
    pending = []
    for ci, m0 in enumerate(chunks):
        ps_lo = psp.tile([128, 515], f32, tag="psL", bufs=2,
                         name=f"psL_{sink.tag}_{ci}")
        ps_hi = psp.tile([128, 515], f32, tag="psH", bufs=2,
                         name=f"psH_{sink.tag}_{ci}")
        for f, ps in ((0, ps_lo), (1, ps_hi)):
            for si, (o, j0, j1) in enumerate(slots):
                lhsT = lhsT_of(si, m0)
                for (a, b) in _split_512(j0, j1):
                    w0 = a - j0
                    nc.tensor.matmul(ps[0:128, a:b], lhsT,
                                     wsb[:, si, f, w0:w0 + (b - a)],
                                     start=True, stop=True)
            # drain one quadrant-group behind the matmuls: by the time the
            # copy decodes, its matmuls have finished (ACT has no exec queue)
            pending.append((ci, f, ps))
            if len(pending) > 1:
                sink(*pending.pop(0), copy_engines)
    for item in pending:
        sink(*item, copy_engines)


def _dt():
    import concourse.mybir as mybir
    return mybir.dt


class _EngRot:
    """round-robin copy chooser, weighted toward the faster engines."""

    def __init__(self, nc):
        def act(out, in_):
            nc.scalar.copy(out=out, in_=in_)

        def dve(out, in_):
            nc.vector.tensor_copy(out=out, in_=in_)

        self.seq = [act, dve]
        self.i = 0

    def next(self):
        e = self.seq[self.i % len(self.seq)]
        self.i += 1
        return e


def build_bass(n_images=IMGS_PER_CORE, repeats=1):
    import concourse.mybir as mybir
    import concourse.tile as tile
    from concourse import bacc
    from concourse.ap import AP
    from contextlib import ExitStack

    nc = bacc.Bacc("TRN2", target_bir_lowering=False, debug=False)

    xin = nc.dram_tensor("xin", (n_images, H, W), mybir.dt.float32,
                         kind="ExternalInput").ap()
    out = nc.dram_tensor("out", (n_images, IMG_ELEMS), mybir.dt.float32,
                         kind="ExternalOutput").ap()
    wdram = {N: nc.dram_tensor(f"w{N}", WEIGHTS[N].shape, mybir.dt.float16,
                               kind="ExternalInput").ap() for N in LEVEL_NS}

    with tile.TileContext(nc) as tc, ExitStack() as ctx:
        cpool = ctx.enter_context(tc.tile_pool(name="consts", bufs=1))
        apool = ctx.enter_context(tc.tile_pool(name="act", bufs=1))
        stpool = ctx.enter_context(tc.tile_pool(name="stage", bufs=1))
        psp = ctx.enter_context(tc.tile_pool(name="ps", bufs=1, space="PSUM"))

        wsb = {}
        for N in LEVEL_NS:
            wsb[N] = cpool.tile(list(WEIGHTS[N].shape), mybir.dt.float16,
                                name=f"wsb{N}")
            nc.sync.dma_start(out=wsb[N][:], in_=wdram[N])

        rot = _EngRot(nc)
        for _rep in range(repeats):
            # staggered wavefront: image i runs pass p at step i + p, so each
            # op only waits on ops emitted at earlier steps (no head-of-line
            # deadlock) and every engine sees a uniform mix of work.
            plans = [_image_passes(nc, apool, stpool, psp, wsb, xin, out,
                                   img, rot, AP) for img in range(n_images)]
            for step in range(n_images + 5):
                for img in range(n_images):
                    p = step - img
                    if 0 <= p < 6:
                        plans[img][p]()

    nc.compile()
    return nc


class _P1Sink:
    """p1: psum [128, 2Np] -> B[:, ci, :] (fp16)."""

    def __init__(self, nc, Np, Btile, tag):
        self.nc, self.Np, self.B, self.tag = nc, Np, Btile, tag
        self.chunks = [o for o, _, _ in SLOTS[{515: 1024, 261: 515,
                                               134: 261}[Np]]]

    def __call__(self, ci, f, ps, rot):
        Np = self.Np
        rot.next()(self.B[:, ci, f * Np:(f + 1) * Np], ps[:, 0:Np])


class _P2Sink:
    """p2: psum -> next-level A (fp16) + fp32 det staging tiles; DMAs are
    emitted by the caller once the stage tiles fill."""

    def __init__(self, nc, N, A_next, st_lo_hi, st_hi_lo, st_hi_hi, st_lo_lo):
        # st_lo_hi = cV stage (lo chunks, hi cols); st_hi_lo = cH;
        # st_hi_hi = cD; st_lo_lo = cA3 stage (only for the last level).
        self.nc = nc
        self.Np = nprime(N)
        self.A_next, self.cv, self.ch, self.cd = A_next, st_lo_hi, st_hi_lo, st_hi_hi
        self.ca = st_lo_lo
        self.lo = P2_LO[N]
        self.hi = P2_HI[N]
        self.chunks = self.lo + self.hi
        self.N = N

    def __call__(self, ci, f, ps, rot):
        Np = self.Np
        if ci < len(self.lo):
            if f == 0:
                dst = self.A_next if self.A_next is not None else self.ca
                rot.next()(dst[:, ci, :], ps[:, 0:Np])
            else:
                rot.next()(self.cv[:, ci, :], ps[:, 0:Np])
        else:
            hc = ci - len(self.lo)
            if f == 0:
                rot.next()(self.ch[:, hc, :], ps[:, 0:Np])
            elif not (self.N == 1024 and hc == len(self.hi) - 1):
                # cD1's last slot is fully covered by slot 3 (truncation)
                rot.next()(self.cd[:, hc, :], ps[:, 0:Np])


def _emit_det_dmas(nc, out, img, name, stage, row_offs, AP):
    """DMA a staged detail section (overlapping slot layout) to DRAM."""
    sec_base, Wd = SECT[name]
    eng = nc.sync
    base = img * IMG_ELEMS + sec_base
    n_slots = len(row_offs)
    if name == "cD1":
        # slots rows {0,128,256,384,387}; valid: 469 full rows + 404 cols
        dst = AP(out.tensor, base, [[Wd, 128], [128 * Wd, 3], [1, Wd]])
        eng.dma_start(out=dst, in_=stage[:, 0:3, :])
        dst = AP(out.tensor, base + 384 * Wd, [[Wd, 85], [1, Wd]])
        eng.dma_start(out=dst, in_=stage[0:85, 3, :])
        dst = AP(out.tensor, base + CD1_FULL_ROWS * Wd, [[1, CD1_PART_COLS]])
        eng.dma_start(out=dst, in_=stage[85:86, 3, 0:CD1_PART_COLS])
        return
    if n_slots == 2 and row_offs[1] - row_offs[0] < 128:
        # one overlapping-dest DMA covers everything (L3 sections); the
        # rewritten rows cost ~4us of DMA but splitting into two DMAs costs
        # more in SP/HWDGE serialization (measured)
        step = row_offs[1] - row_offs[0]
        dst = AP(out.tensor, base, [[Wd, 128], [step * Wd, 2], [1, Wd]])
        eng.dma_start(out=dst, in_=stage[:, 0:2, :])
        return
    # uniform prefix + fresh tail
    step = row_offs[1] - row_offs[0]
    nu = n_slots - 1
    dst = AP(out.tensor, base, [[Wd, 128], [step * Wd, nu], [1, Wd]])
    eng.dma_start(out=dst, in_=stage[:, 0:nu, :])
    o_last = row_offs[-1]
    fresh0 = row_offs[-2] + 128          # first row not covered by prefix
    p0 = fresh0 - o_last
    npart = o_last + 128 - fresh0
    dst = AP(out.tensor, base + fresh0 * Wd, [[Wd, npart], [1, Wd]])
    eng.dma_start(out=dst, in_=stage[p0:p0 + npart, n_slots - 1, :])


def _image_passes(nc, apool, stpool, psp, wsb, xin, out, img, rot, AP):
    """Allocate the image's tiles, emit its input DMAs, and return the six
    pass thunks (p1a, p2a, p1b, p2b, p1c, p2c) for pairwise interleaving."""
    import concourse.mybir as mybir
    f16, f32 = mybir.dt.float16, mybir.dt.float32

    # input: cast DMA into overlapping h-slots
    A1 = apool.tile([128, 9, 1024], f16, tag="A1", bufs=AB_BUFS, name=f"A1_{img}")
    src = AP(xin.tensor, img * H * W, [[W, 128], [122 * W, 4], [1, W]])
    nc.gpsimd.dma_start(out=A1[:, 0:4, :], in_=src)
    src = AP(xin.tensor, (img * H + 488) * W, [[W, 128], [122 * W, 4], [1, W]])
    nc.gpsimd.dma_start(out=A1[:, 4:8, :], in_=src)
    nc.gpsimd.dma_start(out=A1[:, 8, :], in_=xin[img, 896:1024, :])

    B1 = apool.tile([128, 9, 1030], f16, tag="B1", bufs=AB_BUFS, name=f"B1_{img}")
    A2 = apool.tile([128, 5, 515], f16, tag="A2", bufs=AB_BUFS, name=f"A2_{img}")
    B2 = apool.tile([128, 5, 522], f16, tag="B2", bufs=AB_BUFS, name=f"B2_{img}")
    A3 = apool.tile([128, 3, 261], f16, tag="A3", bufs=AB_BUFS, name=f"A3_{img}")
    B3 = apool.tile([128, 3, 268], f16, tag="B3", bufs=AB_BUFS, name=f"B3_{img}")

    def p1a():
        s = _P1Sink(nc, 515, B1, f"p1a_{img}")
        _emit_pass(nc, psp, 1024, lambda si, m0: A1[:, si, m0:m0 + 128],
                   wsb[1024], s, rot)

    def p2a():
        cv1 = stpool.tile([128, 5, 515], f32, tag="st515", bufs=ST515_BUFS,
                          name=f"cv1_{img}")
        ch1 = stpool.tile([128, 5, 515], f32, tag="st515", bufs=ST515_BUFS,
                          name=f"ch1_{img}")
        cd1 = stpool.tile([128, 5, 515], f32, tag="st515", bufs=ST515_BUFS,
                          name=f"cd1_{img}")
        s = _P2Sink(nc, 1024, A2, cv1, ch1, cd1, None)
        s.tag = f"p2a_{img}"
        _emit_pass(nc, psp, 1024, lambda si, m0: B1[:, si, m0:m0 + 128],
                   wsb[1024], s, rot)
        _emit_det_dmas(nc, out, img, "cV1", cv1, [0, 122, 244, 366, 387], AP)
        _emit_det_dmas(nc, out, img, "cH1", ch1, [0, 128, 256, 384, 387], AP)
        _emit_det_dmas(nc, out, img, "cD1", cd1, [0, 128, 256, 384, 387], AP)

    def p1b():
        s = _P1Sink(nc, 261, B2, f"p1b_{img}")
        _emit_pass(nc, psp, 515, lambda si, m0: A2[:, si, m0:m0 + 128],
                   wsb[515], s, rot)

    def p2b():
        cv2 = stpool.tile([128, 3, 261], f32, tag="st261", bufs=6,
                          name=f"cv2_{img}")
        ch2 = stpool.tile([128, 3, 261], f32, tag="st261", bufs=6,
                          name=f"ch2_{img}")
        cd2 = stpool.tile([128, 3, 261], f32, tag="st261", bufs=6,
                          name=f"cd2_{img}")
        s = _P2Sink(nc, 515, A3, cv2, ch2, cd2, None)
        s.tag = f"p2b_{img}"
        _emit_pass(nc, psp, 515, lambda si, m0: B2[:, si, m0:m0 + 128],
                   wsb[515], s, rot)
        _emit_det_dmas(nc, out, img, "cV2", cv2, [0, 122, 133], AP)
        _emit_det_dmas(nc, out, img, "cH2", ch2, [0, 128, 133], AP)
        _emit_det_dmas(nc, out, img, "cD2", cd2, [0, 128, 133], AP)

    def p1c():
        s = _P1Sink(nc, 134, B3, f"p1c_{img}")
        _emit_pass(nc, psp, 261, lambda si, m0: A3[:, si, m0:m0 + 128],
                   wsb[261], s, rot)

    def p2c():
        ca3 = stpool.tile([128, 2, 134], f32, tag="st134", bufs=8,
                          name=f"ca3_{img}")
        cv3 = stpool.tile([128, 2, 134], f32, tag="st134", bufs=8,
                          name=f"cv3_{img}")
        ch3 = stpool.tile([128, 2, 134], f32, tag="st134", bufs=8,
                          name=f"ch3_{img}")
        cd3 = stpool.tile([128, 2, 134], f32, tag="st134", bufs=8,
                          name=f"cd3_{img}")
        s = _P2Sink(nc, 261, None, cv3, ch3, cd3, ca3)
        s.tag = f"p2c_{img}"
        _emit_pass(nc, psp, 261, lambda si, m0: B3[:, si, m0:m0 + 128],
                   wsb[261], s, rot)
        _emit_det_dmas(nc, out, img, "cA3", ca3, [0, 6], AP)
        _emit_det_dmas(nc, out, img, "cV3", cv3, [0, 6], AP)
        _emit_det_dmas(nc, out, img, "cH3", ch3, [0, 6], AP)
        _emit_det_dmas(nc, out, img, "cD3", cd3, [0, 6], AP)

    return [p1a, p2a, p1b, p2b, p1c, p2c]


# ----------------------------------------------------------------- runner
EXTRA_INPUTS = {f"w{N}": WEIGHTS[N] for N in LEVEL_NS}


def _get_built():
    global _BUILT
    if _BUILT is None:
        _BUILT = build_bass()
    return _BUILT


def kernel(x: np.ndarray) -> np.ndarray:
    from concourse import bass_utils

    x = np.ascontiguousarray(np.asarray(x), dtype=np.float32)
    assert x.shape == (B, C, H, W), x.shape
    nc = _get_built()

    imgs = x.reshape(B * C, H, W)
    in_maps = []
    for c in range(N_CORES):
        m = {"xin": imgs[c * IMGS_PER_CORE:(c + 1) * IMGS_PER_CORE]}
        m.update(EXTRA_INPUTS)
        in_maps.append(m)

    res = bass_utils.run_bass_kernel_spmd(nc, in_maps,
                                          core_ids=list(range(N_CORES)))
    outs = [res.results[c]["out"] for c in range(N_CORES)]
    flat = np.concatenate(outs, axis=0)  # [48, 1048576]
    return flat.reshape(B, C, 64, 128, 128)


# revision 32
# speedup vs baseline: 1.0435x; 1.0435x over previous
"""Trainium2 Bass kernel for nn_DWT_Layer: 3-level 2D db4 DWT (symmetric mode).

Input  x: (16, 3, 1024, 1024) fp32.
Output:   (16, 3, 64, 128, 128) fp32 — the flattened/truncated wavelet pyramid
          [cA3, cH3, cV3, cD3, cH2, cV2, cD2, cH1, cV1, cD1(truncated)].

Sharding: pure data parallel — 48 (batch*channel) images, 6 per core on 8
NeuronCores, no communication.

Algorithm (all-PE, transpose-free): each 1D DWT pass along the partition
axis is a set of banded fp16 matmuls out[m,j] = sum_r A[r,m]*M2[j,r] with
the DATA as lhsT and the folded/stacked DWT band matrix as rhs. The
contraction rows are stored in overlapping 128-row "slots" (stride <=122)
so that every output row j is owned by exactly one slot -> each psum
column is written by a single start=stop matmul (no accumulation, no
pre-zeroing) and the output comes out transposed. Running the same pass
twice (height then width) returns to row-major orientation, so the whole
3-level pyramid needs zero transposes, zero DVE MAC chains and zero
mirror ops: just cast-DMAs in, banded matmuls, PSUM->SBUF copies
(fp32->fp16 for the next stage / fp32 for detail staging) and row DMAs
out.
"""
import numpy as np

# ----------------------------------------------------------------- constants
DEC_LO = np.array([-0.010597401784997278, 0.032883011666982945,
                   0.030841381835986965, -0.18703481171888114,
                   -0.027983769416983849, 0.63088076792959036,
                   0.71484657055254153, 0.23037781330885523], dtype=np.float64)
L = 8
DEC_HI = np.array([(-1.0) ** (k + 1) * DEC_LO[L - 1 - k] for k in range(L)],
                  dtype=np.float64)

B, C, H, W = 16, 3, 1024, 1024
N_CORES = 8
IMGS_PER_CORE = 6
IMG_ELEMS = H * W

LEVEL_NS = [1024, 515, 261]   # input edge length per level


def nprime(N):
    return (N + 5) // 2 + 1


def ext_index(j, N):
    if j < 6:
        return 5 - j
    if j < N + 6:
        return j - 6
    return 2 * N + 5 - j


def dwt_matrix(N, filt):
    Np = nprime(N)
    M = np.zeros((Np, N), dtype=np.float64)
    filtrev = filt[::-1]
    for i in range(Np):
        for t in range(L):
            M[i, ext_index(2 * i + t, N)] += filtrev[t]
    return M


def build_slots(N):
    """[(o, j0, j1)]: slot covers input rows [o, o+128); owns outputs
    [j0, j1) whose (lo and hi) supports lie inside the slot."""
    Np = nprime(N)
    Mlo = dwt_matrix(N, DEC_LO)
    Mhi = dwt_matrix(N, DEC_HI)
    lo_r, hi_r = [], []
    for j in range(Np):
        nz = np.nonzero(np.abs(Mlo[j]) + np.abs(Mhi[j]))[0]
        lo_r.append(int(nz.min()))
        hi_r.append(int(nz.max()))
    slots = []
    j = 0
    while j < Np:
        o = min(max(0, lo_r[j]), N - 128)
        j1 = j
        while j1 < Np and lo_r[j1] >= o and hi_r[j1] < o + 128:
            j1 += 1
        assert j1 > j
        slots.append((o, j, j1))
        j = j1
    return slots


BW = 64  # max band width per slot


def build_weights(N):
    """fp16 packed rhs bands: [128, n_slots, 2, BW]."""
    slots = build_slots(N)
    Mlo = dwt_matrix(N, DEC_LO)
    Mhi = dwt_matrix(N, DEC_HI)
    arr = np.zeros((128, len(slots), 2, BW), dtype=np.float16)
    for s, (o, j0, j1) in enumerate(slots):
        assert j1 - j0 <= BW
        arr[:, s, 0, :j1 - j0] = Mlo[j0:j1, o:o + 128].T
        arr[:, s, 1, :j1 - j0] = Mhi[j0:j1, o:o + 128].T
    return arr


SLOTS = {N: build_slots(N) for N in LEVEL_NS}
WEIGHTS = {N: build_weights(N) for N in LEVEL_NS}

# p2 m-chunk offsets (within the 2Np free axis of B) per level:
#   lo chunks = next level's A slots (aa feeds them 1:1); hi chunks cover the
#   hi half with non-overlapping 128s + one 128-tail.
P2_LO = {1024: [o for o, _, _ in SLOTS[515]],   # {0,122,244,366,387}
         515: [o for o, _, _ in SLOTS[261]],    # {0,122,133}
         261: [0, 6]}
P2_HI = {1024: [515 + r for r in (0, 128, 256, 384, 387)],
         515: [261 + r for r in (0, 128, 133)],
         261: [134 + r for r in (0, 6)]}

# output section offsets (elements within one image's 1048576-long output)
SECT = {}
_cur = 0
for _name, _n in [("cA3", 134), ("cH3", 134), ("cV3", 134), ("cD3", 134),
                  ("cH2", 261), ("cV2", 261), ("cD2", 261),
                  ("cH1", 515), ("cV1", 515), ("cD1", 515)]:
    SECT[_name] = (_cur, _n)
    _cur += _n * _n
CD1_FULL_ROWS = 469
CD1_PART_COLS = 404
assert SECT["cD1"][0] + CD1_FULL_ROWS * 515 + CD1_PART_COLS == IMG_ELEMS

AB_BUFS = 2      # buffers for the A/B activation tiles
ST515_BUFS = 6   # fp32 detail-staging buffers (3 live per image)

_BUILT = None


def _split_512(a, b):
    """split [a, b) at multiples of 512 (psum bank boundaries)."""
    out = []
    while a < b:
        e = min(b, (a // 512 + 1) * 512)
        out.append((a, e))
        a = e
    return out


def _emit_pass(nc, psp, N, lhsT_of, wsb, sink, copy_engines):
    """One DWT pass: for each m-chunk, banded matmuls into two psum tiles
    (independent lo/hi pipelining lanes), then sink(ci, (lo, hi)) emits the
    copies out of psum."""
    Np = nprime(N)
    slots = SLOTS[N]
    chunks = sink.chunks
    f32 = _dt().float32
    fused = 2 * Np <= 522  # L2/L3: both quadrants fit one psum tile
    pending = []

    def drain(item):
        ci_, f_, ps_, c0_ = item
        sink(ci_, f_, ps_, c0_, copy_engines)

    for ci, m0 in enumerate(chunks):
        if fused:
            # alternate fused chunks across both psum lanes to keep the
            # two FIFOs balanced
            tag = "psL" if ci % 2 == 0 else "psH"
            ps_lo = psp.tile([128, 522], f32, tag=tag, bufs=2,
                             name=f"ps_{sink.tag}_{ci}")
            ps_hi = ps_lo
        else:
            ps_lo = psp.tile([128, 522], f32, tag="psL", bufs=2,
                             name=f"psL_{sink.tag}_{ci}")
            ps_hi = psp.tile([128, 522], f32, tag="psH", bufs=2,
                             name=f"psH_{sink.tag}_{ci}")
        for f, ps, c0 in ((0, ps_lo, 0), (1, ps_hi, Np if fused else 0)):
            for si, (o, j0, j1) in enumerate(slots):
                lhsT = lhsT_of(si, m0)
                for (a, b) in _split_512(c0 + j0, c0 + j1):
                    w0 = a - c0 - j0
                    nc.tensor.matmul(ps[0:128, a:b], lhsT,
                                     wsb[:, si, f, w0:w0 + (b - a)],
                                     start=True, stop=True)
            # drain one quadrant-group behind the matmuls: by the time the
            # copy decodes, its matmuls have finished (ACT has no exec queue)
            if not (fused and f == 0 and sink.merges_fused):
                pending.append((ci, f, ps, c0))
            if len(pending) > 1:
                drain(pending.pop(0))
    for item in pending:
        drain(item)


def _dt():
    import concourse.mybir as mybir
    return mybir.dt


class _EngRot:
    """round-robin copy chooser, weighted toward the faster engines."""

    def __init__(self, nc):
        def act(out, in_):
            nc.scalar.copy(out=out, in_=in_)

        def dve(out, in_):
            nc.vector.tensor_copy(out=out, in_=in_)

        self.seq = [act, dve]
        self.i = 0

    def next(self):
        e = self.seq[self.i % len(self.seq)]
        self.i += 1
        return e


def build_bass(n_images=IMGS_PER_CORE, repeats=1):
    import concourse.mybir as mybir
    import concourse.tile as tile
    from concourse import bacc
    from concourse.ap import AP
    from contextlib import ExitStack

    nc = bacc.Bacc("TRN2", target_bir_lowering=False, debug=False)

    xin = nc.dram_tensor("xin", (n_images, H, W), mybir.dt.float32,
                         kind="ExternalInput").ap()
    out = nc.dram_tensor("out", (n_images, IMG_ELEMS), mybir.dt.float32,
                         kind="ExternalOutput").ap()
    wdram = {N: nc.dram_tensor(f"w{N}", WEIGHTS[N].shape, mybir.dt.float16,
                               kind="ExternalInput").ap() for N in LEVEL_NS}

    with tile.TileContext(nc) as tc, ExitStack() as ctx:
        cpool = ctx.enter_context(tc.tile_pool(name="consts", bufs=1))
        apool = ctx.enter_context(tc.tile_pool(name="act", bufs=1))
        stpool = ctx.enter_context(tc.tile_pool(name="stage", bufs=1))
        psp = ctx.enter_context(tc.tile_pool(name="ps", bufs=1, space="PSUM"))

        wsb = {}
        for N in LEVEL_NS:
            wsb[N] = cpool.tile(list(WEIGHTS[N].shape), mybir.dt.float16,
                                name=f"wsb{N}")
            nc.sync.dma_start(out=wsb[N][:], in_=wdram[N])

        rot = _EngRot(nc)
        for _rep in range(repeats):
            # staggered wavefront: image i runs pass p at step i + p, so each
            # op only waits on ops emitted at earlier steps (no head-of-line
            # deadlock) and every engine sees a uniform mix of work.
            plans = [_image_passes(nc, apool, stpool, psp, wsb, xin, out,
                                   img, rot, AP) for img in range(n_images)]
            for step in range(n_images + 5):
                for img in range(n_images):
                    p = step - img
                    if 0 <= p < 6:
                        plans[img][p]()

    nc.compile()
    return nc


class _P1Sink:
    """p1: psum [128, 2Np] -> B[:, ci, :] (fp16)."""

    def __init__(self, nc, Np, Btile, tag):
        self.nc, self.Np, self.B, self.tag = nc, Np, Btile, tag
        self.chunks = [o for o, _, _ in SLOTS[{515: 1024, 261: 515,
                                               134: 261}[Np]]]

    merges_fused = True

    def __call__(self, ci, f, ps, c0, rot):
        Np = self.Np
        if c0 == Np:  # fused tile: one copy moves both quadrants
            rot.next()(self.B[:, ci, 0:2 * Np], ps[:, 0:2 * Np])
        else:
            rot.next()(self.B[:, ci, f * Np:(f + 1) * Np], ps[:, 0:Np])


class _P2Sink:
    """p2: psum -> next-level A (fp16) + fp32 det staging tiles; DMAs are
    emitted by the caller once the stage tiles fill."""

    def __init__(self, nc, N, A_next, st_lo_hi, st_hi_lo, st_hi_hi, st_lo_lo):
        # st_lo_hi = cV stage (lo chunks, hi cols); st_hi_lo = cH;
        # st_hi_hi = cD; st_lo_lo = cA3 stage (only for the last level).
        self.nc = nc
        self.Np = nprime(N)
        self.A_next, self.cv, self.ch, self.cd = A_next, st_lo_hi, st_hi_lo, st_hi_hi
        self.ca = st_lo_lo
        self.lo = P2_LO[N]
        self.hi = P2_HI[N]
        self.chunks = self.lo + self.hi
        self.N = N

    merges_fused = False

    def __call__(self, ci, f, ps, c0, rot):
        Np = self.Np
        ps = ps[:, c0:c0 + Np] if c0 else ps
        if ci < len(self.lo):
            if f == 0:
                dst = self.A_next if self.A_next is not None else self.ca
                rot.next()(dst[:, ci, :], ps[:, 0:Np])
            else:
                rot.next()(self.cv[:, ci, :], ps[:, 0:Np])
        else:
            hc = ci - len(self.lo)
            if f == 0:
                rot.next()(self.ch[:, hc, :], ps[:, 0:Np])
            elif not (self.N == 1024 and hc == len(self.hi) - 1):
                # cD1's last slot is fully covered by slot 3 (truncation)
                rot.next()(self.cd[:, hc, :], ps[:, 0:Np])


def _emit_det_dmas(nc, out, img, name, stage, row_offs, AP):
    """DMA a staged detail section (overlapping slot layout) to DRAM."""
    sec_base, Wd = SECT[name]
    eng = nc.sync
    base = img * IMG_ELEMS + sec_base
    n_slots = len(row_offs)
    if name == "cD1":
        # slots rows {0,128,256,384,387}; valid: 469 full rows + 404 cols
        dst = AP(out.tensor, base, [[Wd, 128], [128 * Wd, 3], [1, Wd]])
        eng.dma_start(out=dst, in_=stage[:, 0:3, :])
        dst = AP(out.tensor, base + 384 * Wd, [[Wd, 85], [1, Wd]])
        eng.dma_start(out=dst, in_=stage[0:85, 3, :])
        dst = AP(out.tensor, base + CD1_FULL_ROWS * Wd, [[1, CD1_PART_COLS]])
        eng.dma_start(out=dst, in_=stage[85:86, 3, 0:CD1_PART_COLS])
        return
    if n_slots == 2 and row_offs[1] - row_offs[0] < 128:
        # one overlapping-dest DMA covers everything (L3 sections); the
        # rewritten rows cost ~4us of DMA but splitting into two DMAs costs
        # more in SP/HWDGE serialization (measured)
        step = row_offs[1] - row_offs[0]
        dst = AP(out.tensor, base, [[Wd, 128], [step * Wd, 2], [1, Wd]])
        eng.dma_start(out=dst, in_=stage[:, 0:2, :])
        return
    # uniform prefix + fresh tail
    step = row_offs[1] - row_offs[0]
    nu = n_slots - 1
    dst = AP(out.tensor, base, [[Wd, 128], [step * Wd, nu], [1, Wd]])
    eng.dma_start(out=dst, in_=stage[:, 0:nu, :])
    o_last = row_offs[-1]
    fresh0 = row_offs[-2] + 128          # first row not covered by prefix
    p0 = fresh0 - o_last
    npart = o_last + 128 - fresh0
    dst = AP(out.tensor, base + fresh0 * Wd, [[Wd, npart], [1, Wd]])
    eng.dma_start(out=dst, in_=stage[p0:p0 + npart, n_slots - 1, :])


def _image_passes(nc, apool, stpool, psp, wsb, xin, out, img, rot, AP):
    """Allocate the image's tiles, emit its input DMAs, and return the six
    pass thunks (p1a, p2a, p1b, p2b, p1c, p2c) for pairwise interleaving."""
    import concourse.mybir as mybir
    f16, f32 = mybir.dt.float16, mybir.dt.float32

    # input: cast DMA into overlapping h-slots
    A1 = apool.tile([128, 9, 1024], f16, tag="A1", bufs=AB_BUFS, name=f"A1_{img}")
    src = AP(xin.tensor, img * H * W, [[W, 128], [122 * W, 4], [1, W]])
    nc.gpsimd.dma_start(out=A1[:, 0:4, :], in_=src)
    src = AP(xin.tensor, (img * H + 488) * W, [[W, 128], [122 * W, 4], [1, W]])
    nc.gpsimd.dma_start(out=A1[:, 4:8, :], in_=src)
    nc.gpsimd.dma_start(out=A1[:, 8, :], in_=xin[img, 896:1024, :])

    B1 = apool.tile([128, 9, 1030], f16, tag="B1", bufs=AB_BUFS, name=f"B1_{img}")
    A2 = apool.tile([128, 5, 515], f16, tag="A2", bufs=AB_BUFS, name=f"A2_{img}")
    B2 = apool.tile([128, 5, 522], f16, tag="B2", bufs=AB_BUFS, name=f"B2_{img}")
    A3 = apool.tile([128, 3, 261], f16, tag="A3", bufs=AB_BUFS, name=f"A3_{img}")
    B3 = apool.tile([128, 3, 268], f16, tag="B3", bufs=AB_BUFS, name=f"B3_{img}")

    def p1a():
        s = _P1Sink(nc, 515, B1, f"p1a_{img}")
        _emit_pass(nc, psp, 1024, lambda si, m0: A1[:, si, m0:m0 + 128],
                   wsb[1024], s, rot)

    def p2a():
        cv1 = stpool.tile([128, 5, 515], f32, tag="st515", bufs=ST515_BUFS,
                          name=f"cv1_{img}")
        ch1 = stpool.tile([128, 5, 515], f32, tag="st515", bufs=ST515_BUFS,
                          name=f"ch1_{img}")
        cd1 = stpool.tile([128, 5, 515], f32, tag="st515", bufs=ST515_BUFS,
                          name=f"cd1_{img}")
        s = _P2Sink(nc, 1024, A2, cv1, ch1, cd1, None)
        s.tag = f"p2a_{img}"
        _emit_pass(nc, psp, 1024, lambda si, m0: B1[:, si, m0:m0 + 128],
                   wsb[1024], s, rot)
        _emit_det_dmas(nc, out, img, "cV1", cv1, [0, 122, 244, 366, 387], AP)
        _emit_det_dmas(nc, out, img, "cH1", ch1, [0, 128, 256, 384, 387], AP)
        _emit_det_dmas(nc, out, img, "cD1", cd1, [0, 128, 256, 384, 387], AP)

    def p1b():
        s = _P1Sink(nc, 261, B2, f"p1b_{img}")
        _emit_pass(nc, psp, 515, lambda si, m0: A2[:, si, m0:m0 + 128],
                   wsb[515], s, rot)

    def p2b():
        cv2 = stpool.tile([128, 3, 261], f32, tag="st261", bufs=6,
                          name=f"cv2_{img}")
        ch2 = stpool.tile([128, 3, 261], f32, tag="st261", bufs=6,
                          name=f"ch2_{img}")
        cd2 = stpool.tile([128, 3, 261], f32, tag="st261", bufs=6,
                          name=f"cd2_{img}")
        s = _P2Sink(nc, 515, A3, cv2, ch2, cd2, None)
        s.tag = f"p2b_{img}"
        _emit_pass(nc, psp, 515, lambda si, m0: B2[:, si, m0:m0 + 128],
                   wsb[515], s, rot)
        _emit_det_dmas(nc, out, img, "cV2", cv2, [0, 122, 133], AP)
        _emit_det_dmas(nc, out, img, "cH2", ch2, [0, 128, 133], AP)
        _emit_det_dmas(nc, out, img, "cD2", cd2, [0, 128, 133], AP)

    def p1c():
        s = _P1Sink(nc, 134, B3, f"p1c_{img}")
        _emit_pass(nc, psp, 261, lambda si, m0: A3[:, si, m0:m0 + 128],
                   wsb[261], s, rot)

    def p2c():
        ca3 = stpool.tile([128, 2, 134], f32, tag="st134", bufs=8,
                          name=f"ca3_{img}")
        cv3 = stpool.tile([128, 2, 134], f32, tag="st134", bufs=8,
                          name=f"cv3_{img}")
        ch3 = stpool.tile([128, 2, 134], f32, tag="st134", bufs=8,
                          name=f"ch3_{img}")
        cd3 = stpool.tile([128, 2, 134], f32, tag="st134", bufs=8,
                          name=f"cd3_{img}")
        s = _P2Sink(nc, 261, None, cv3, ch3, cd3, ca3)
        s.tag = f"p2c_{img}"
        _emit_pass(nc, psp, 261, lambda si, m0: B3[:, si, m0:m0 + 128],
                   wsb[261], s, rot)
        _emit_det_dmas(nc, out, img, "cA3", ca3, [0, 6], AP)
        _emit_det_dmas(nc, out, img, "cV3", cv3, [0, 6], AP)
        _emit_det_dmas(nc, out, img, "cH3", ch3, [0, 6], AP)
        _emit_det_dmas(nc, out, img, "cD3", cd3, [0, 6], AP)

    return [p1a, p2a, p1b, p2b, p1c, p2c]


# ----------------------------------------------------------------- runner
EXTRA_INPUTS = {f"w{N}": WEIGHTS[N] for N in LEVEL_NS}


def _get_built():
    global _BUILT
    if _BUILT is None:
        _BUILT = build_bass()
    return _BUILT


def kernel(x: np.ndarray) -> np.ndarray:
    from concourse import bass_utils

    x = np.ascontiguousarray(np.asarray(x), dtype=np.float32)
    assert x.shape == (B, C, H, W), x.shape
    nc = _get_built()

    imgs = x.reshape(B * C, H, W)
    in_maps = []
    for c in range(N_CORES):
        m = {"xin": imgs[c * IMGS_PER_CORE:(c + 1) * IMGS_PER_CORE]}
        m.update(EXTRA_INPUTS)
        in_maps.append(m)

    res = bass_utils.run_bass_kernel_spmd(nc, in_maps,
                                          core_ids=list(range(N_CORES)))
    outs = [res.results[c]["out"] for c in range(N_CORES)]
    flat = np.concatenate(outs, axis=0)  # [48, 1048576]
    return flat.reshape(B, C, 64, 128, 128)


# revision 35
# speedup vs baseline: 1.0444x; 1.0009x over previous
"""Trainium2 Bass kernel for nn_DWT_Layer: 3-level 2D db4 DWT (symmetric mode).

Input  x: (16, 3, 1024, 1024) fp32.
Output:   (16, 3, 64, 128, 128) fp32 — the flattened/truncated wavelet pyramid
          [cA3, cH3, cV3, cD3, cH2, cV2, cD2, cH1, cV1, cD1(truncated)].

Sharding: pure data parallel — 48 (batch*channel) images, 6 per core on 8
NeuronCores, no communication.

Algorithm (all-PE, transpose-free): each 1D DWT pass along the partition
axis is a set of banded fp16 matmuls out[m,j] = sum_r A[r,m]*M2[j,r] with
the DATA as lhsT and the folded/stacked DWT band matrix as rhs. The
contraction rows are stored in overlapping 128-row "slots" (stride <=122)
so that every output row j is owned by exactly one slot -> each psum
column is written by a single start=stop matmul (no accumulation, no
pre-zeroing) and the output comes out transposed. Running the same pass
twice (height then width) returns to row-major orientation, so the whole
3-level pyramid needs zero transposes, zero DVE MAC chains and zero
mirror ops: just cast-DMAs in, banded matmuls, PSUM->SBUF copies
(fp32->fp16 for the next stage / fp32 for detail staging) and row DMAs
out.
"""
import numpy as np

# ----------------------------------------------------------------- constants
DEC_LO = np.array([-0.010597401784997278, 0.032883011666982945,
                   0.030841381835986965, -0.18703481171888114,
                   -0.027983769416983849, 0.63088076792959036,
                   0.71484657055254153, 0.23037781330885523], dtype=np.float64)
L = 8
DEC_HI = np.array([(-1.0) ** (k + 1) * DEC_LO[L - 1 - k] for k in range(L)],
                  dtype=np.float64)

B, C, H, W = 16, 3, 1024, 1024
N_CORES = 8
IMGS_PER_CORE = 6
IMG_ELEMS = H * W

LEVEL_NS = [1024, 515, 261]   # input edge length per level


def nprime(N):
    return (N + 5) // 2 + 1


def ext_index(j, N):
    if j < 6:
        return 5 - j
    if j < N + 6:
        return j - 6
    return 2 * N + 5 - j


def dwt_matrix(N, filt):
    Np = nprime(N)
    M = np.zeros((Np, N), dtype=np.float64)
    filtrev = filt[::-1]
    for i in range(Np):
        for t in range(L):
            M[i, ext_index(2 * i + t, N)] += filtrev[t]
    return M


def build_slots(N):
    """[(o, j0, j1)]: slot covers input rows [o, o+128); owns outputs
    [j0, j1) whose (lo and hi) supports lie inside the slot."""
    Np = nprime(N)
    Mlo = dwt_matrix(N, DEC_LO)
    Mhi = dwt_matrix(N, DEC_HI)
    lo_r, hi_r = [], []
    for j in range(Np):
        nz = np.nonzero(np.abs(Mlo[j]) + np.abs(Mhi[j]))[0]
        lo_r.append(int(nz.min()))
        hi_r.append(int(nz.max()))
    slots = []
    j = 0
    while j < Np:
        o = min(max(0, lo_r[j]), N - 128)
        j1 = j
        while j1 < Np and lo_r[j1] >= o and hi_r[j1] < o + 128:
            j1 += 1
        assert j1 > j
        slots.append((o, j, j1))
        j = j1
    return slots


BW = 64  # max band width per slot


def build_weights(N):
    """fp16 packed rhs bands: [128, n_slots, 2, BW]."""
    slots = build_slots(N)
    Mlo = dwt_matrix(N, DEC_LO)
    Mhi = dwt_matrix(N, DEC_HI)
    arr = np.zeros((128, len(slots), 2, BW), dtype=np.float16)
    for s, (o, j0, j1) in enumerate(slots):
        assert j1 - j0 <= BW
        arr[:, s, 0, :j1 - j0] = Mlo[j0:j1, o:o + 128].T
        arr[:, s, 1, :j1 - j0] = Mhi[j0:j1, o:o + 128].T
    return arr


SLOTS = {N: build_slots(N) for N in LEVEL_NS}
WEIGHTS = {N: build_weights(N) for N in LEVEL_NS}

# p2 m-chunk offsets (within the 2Np free axis of B) per level:
#   lo chunks = next level's A slots (aa feeds them 1:1); hi chunks cover the
#   hi half with non-overlapping 128s + one 128-tail.
P2_LO = {1024: [o for o, _, _ in SLOTS[515]],   # {0,122,244,366,387}
         515: [o for o, _, _ in SLOTS[261]],    # {0,122,133}
         261: [0, 6]}
P2_HI = {1024: [515 + r for r in (0, 128, 256, 384, 387)],
         515: [261 + r for r in (0, 128, 133)],
         261: [134 + r for r in (0, 6)]}

# output section offsets (elements within one image's 1048576-long output)
SECT = {}
_cur = 0
for _name, _n in [("cA3", 134), ("cH3", 134), ("cV3", 134), ("cD3", 134),
                  ("cH2", 261), ("cV2", 261), ("cD2", 261),
                  ("cH1", 515), ("cV1", 515), ("cD1", 515)]:
    SECT[_name] = (_cur, _n)
    _cur += _n * _n
CD1_FULL_ROWS = 469
CD1_PART_COLS = 404
assert SECT["cD1"][0] + CD1_FULL_ROWS * 515 + CD1_PART_COLS == IMG_ELEMS

AB_BUFS = 2      # buffers for the A/B activation tiles
ST515_BUFS = 6   # fp32 detail-staging buffers (3 live per image)

_BUILT = None


def _split_512(a, b):
    """split [a, b) at multiples of 512 (psum bank boundaries)."""
    out = []
    while a < b:
        e = min(b, (a // 512 + 1) * 512)
        out.append((a, e))
        a = e
    return out


def _emit_pass(nc, psp, N, lhsT_of, wsb, sink, copy_engines):
    """One DWT pass: for each m-chunk, banded matmuls into two psum tiles
    (independent lo/hi pipelining lanes), then sink(ci, (lo, hi)) emits the
    copies out of psum."""
    Np = nprime(N)
    slots = SLOTS[N]
    chunks = sink.chunks
    f32 = _dt().float32
    fused = 2 * Np <= 522  # L2/L3: both quadrants fit one psum tile
    pending = []

    def drain(item):
        ci_, f_, ps_, c0_ = item
        sink(ci_, f_, ps_, c0_, copy_engines)

    for ci, m0 in enumerate(chunks):
        if fused:
            # alternate fused chunks across both psum lanes to keep the
            # two FIFOs balanced
            tag = "psL" if ci % 2 == 0 else "psH"
            ps_lo = psp.tile([128, 522], f32, tag=tag, bufs=2,
                             name=f"ps_{sink.tag}_{ci}")
            ps_hi = ps_lo
        else:
            ps_lo = psp.tile([128, 522], f32, tag="psL", bufs=2,
                             name=f"psL_{sink.tag}_{ci}")
            ps_hi = psp.tile([128, 522], f32, tag="psH", bufs=2,
                             name=f"psH_{sink.tag}_{ci}")
        for f, ps, c0 in ((0, ps_lo, 0), (1, ps_hi, Np if fused else 0)):
            for si, (o, j0, j1) in enumerate(slots):
                lhsT = lhsT_of(si, m0)
                for (a, b) in _split_512(c0 + j0, c0 + j1):
                    w0 = a - c0 - j0
                    nc.tensor.matmul(ps[0:128, a:b], lhsT,
                                     wsb[:, si, f, w0:w0 + (b - a)],
                                     start=True, stop=True)
            # drain one quadrant-group behind the matmuls: by the time the
            # copy decodes, its matmuls have finished (ACT has no exec queue)
            if not (fused and f == 0 and sink.merges_fused):
                pending.append((ci, f, ps, c0))
            if len(pending) > 1:
                drain(pending.pop(0))
    for item in pending:
        drain(item)


def _dt():
    import concourse.mybir as mybir
    return mybir.dt


class _EngRot:
    """round-robin copy chooser, weighted toward the faster engines."""

    def __init__(self, nc):
        def act(out, in_):
            nc.scalar.copy(out=out, in_=in_)

        def dve(out, in_):
            nc.vector.tensor_copy(out=out, in_=in_)

        self.seq = [act, dve]
        self.i = 0

    def next(self):
        e = self.seq[self.i % len(self.seq)]
        self.i += 1
        return e


def build_bass(n_images=IMGS_PER_CORE, repeats=1):
    import concourse.mybir as mybir
    import concourse.tile as tile
    from concourse import bacc
    from concourse.ap import AP
    from contextlib import ExitStack

    nc = bacc.Bacc("TRN2", target_bir_lowering=False, debug=False)

    xin = nc.dram_tensor("xin", (n_images, H, W), mybir.dt.float32,
                         kind="ExternalInput").ap()
    out = nc.dram_tensor("out", (n_images, IMG_ELEMS), mybir.dt.float32,
                         kind="ExternalOutput").ap()
    wdram = {N: nc.dram_tensor(f"w{N}", WEIGHTS[N].shape, mybir.dt.float16,
                               kind="ExternalInput").ap() for N in LEVEL_NS}

    with tile.TileContext(nc) as tc, ExitStack() as ctx:
        cpool = ctx.enter_context(tc.tile_pool(name="consts", bufs=1))
        apool = ctx.enter_context(tc.tile_pool(name="act", bufs=1))
        stpool = ctx.enter_context(tc.tile_pool(name="stage", bufs=1))
        psp = ctx.enter_context(tc.tile_pool(name="ps", bufs=1, space="PSUM"))

        wsb = {}
        for N in LEVEL_NS:
            wsb[N] = cpool.tile(list(WEIGHTS[N].shape), mybir.dt.float16,
                                name=f"wsb{N}")
            nc.sync.dma_start(out=wsb[N][:], in_=wdram[N])

        rot = _EngRot(nc)
        for _rep in range(repeats):
            # staggered wavefront: image i runs pass p at step i + p, so each
            # op only waits on ops emitted at earlier steps (no head-of-line
            # deadlock) and every engine sees a uniform mix of work.
            plans = [_image_passes(nc, apool, stpool, psp, wsb, xin, out,
                                   img, rot, AP) for img in range(n_images)]
            for step in range(n_images + 5):
                for img in range(n_images):
                    p = step - img
                    if 0 <= p < 6:
                        plans[img][p]()

    nc.compile()
    return nc


class _P1Sink:
    """p1: psum [128, 2Np] -> B[:, ci, :] (fp16)."""

    def __init__(self, nc, Np, Btile, tag):
        self.nc, self.Np, self.B, self.tag = nc, Np, Btile, tag
        self.chunks = [o for o, _, _ in SLOTS[{515: 1024, 261: 515,
                                               134: 261}[Np]]]

    merges_fused = True

    def __call__(self, ci, f, ps, c0, rot):
        Np = self.Np
        if c0 == Np:  # fused tile: one copy moves both quadrants
            rot.next()(self.B[:, ci, 0:2 * Np], ps[:, 0:2 * Np])
        else:
            rot.next()(self.B[:, ci, f * Np:(f + 1) * Np], ps[:, 0:Np])


class _P2Sink:
    """p2: psum -> next-level A (fp16) + fp32 det staging tiles; DMAs are
    emitted by the caller once the stage tiles fill."""

    def __init__(self, nc, N, A_next, st_lo_hi, st_hi_lo, st_hi_hi, st_lo_lo):
        # st_lo_hi = cV stage (lo chunks, hi cols); st_hi_lo = cH;
        # st_hi_hi = cD; st_lo_lo = cA3 stage (only for the last level).
        self.nc = nc
        self.Np = nprime(N)
        self.A_next, self.cv, self.ch, self.cd = A_next, st_lo_hi, st_hi_lo, st_hi_hi
        self.ca = st_lo_lo
        self.lo = P2_LO[N]
        self.hi = P2_HI[N]
        self.chunks = self.lo + self.hi
        self.N = N

    merges_fused = False

    def __call__(self, ci, f, ps, c0, rot):
        Np = self.Np
        ps = ps[:, c0:c0 + Np] if c0 else ps
        if ci < len(self.lo):
            if f == 0:
                dst = self.A_next if self.A_next is not None else self.ca
                rot.next()(dst[:, ci, :], ps[:, 0:Np])
            else:
                rot.next()(self.cv[:, ci, :], ps[:, 0:Np])
        else:
            hc = ci - len(self.lo)
            if f == 0:
                rot.next()(self.ch[:, hc, :], ps[:, 0:Np])
            elif not (self.N == 1024 and hc == len(self.hi) - 1):
                # cD1's last slot is fully covered by slot 3 (truncation)
                rot.next()(self.cd[:, hc, :], ps[:, 0:Np])


def _emit_det_dmas(nc, out, img, name, stage, row_offs, AP):
    """DMA a staged detail section (overlapping slot layout) to DRAM."""
    sec_base, Wd = SECT[name]
    eng = nc.sync
    base = img * IMG_ELEMS + sec_base
    n_slots = len(row_offs)
    if name == "cD1":
        # slots rows {0,128,256,384,387}; valid: 469 full rows + 404 cols
        dst = AP(out.tensor, base, [[Wd, 128], [128 * Wd, 3], [1, Wd]])
        eng.dma_start(out=dst, in_=stage[:, 0:3, :])
        dst = AP(out.tensor, base + 384 * Wd, [[Wd, 85], [1, Wd]])
        eng.dma_start(out=dst, in_=stage[0:85, 3, :])
        dst = AP(out.tensor, base + CD1_FULL_ROWS * Wd, [[1, CD1_PART_COLS]])
        eng.dma_start(out=dst, in_=stage[85:86, 3, 0:CD1_PART_COLS])
        return
    if n_slots == 2 and row_offs[1] - row_offs[0] < 128:
        # one overlapping-dest DMA covers everything (L3 sections); the
        # rewritten rows cost ~4us of DMA but splitting into two DMAs costs
        # more in SP/HWDGE serialization (measured)
        step = row_offs[1] - row_offs[0]
        dst = AP(out.tensor, base, [[Wd, 128], [step * Wd, 2], [1, Wd]])
        eng.dma_start(out=dst, in_=stage[:, 0:2, :])
        return
    # uniform prefix + fresh tail
    step = row_offs[1] - row_offs[0]
    nu = n_slots - 1
    dst = AP(out.tensor, base, [[Wd, 128], [step * Wd, nu], [1, Wd]])
    eng.dma_start(out=dst, in_=stage[:, 0:nu, :])
    o_last = row_offs[-1]
    fresh0 = row_offs[-2] + 128          # first row not covered by prefix
    p0 = fresh0 - o_last
    npart = o_last + 128 - fresh0
    dst = AP(out.tensor, base + fresh0 * Wd, [[Wd, npart], [1, Wd]])
    eng.dma_start(out=dst, in_=stage[p0:p0 + npart, n_slots - 1, :])


def _image_passes(nc, apool, stpool, psp, wsb, xin, out, img, rot, AP):
    """Allocate the image's tiles, emit its input DMAs, and return the six
    pass thunks (p1a, p2a, p1b, p2b, p1c, p2c) for pairwise interleaving."""
    import concourse.mybir as mybir
    f16, f32 = mybir.dt.float16, mybir.dt.float32

    # input: cast DMA into overlapping h-slots
    A1 = apool.tile([128, 9, 1024], f16, tag="A1", bufs=AB_BUFS, name=f"A1_{img}")
    src = AP(xin.tensor, img * H * W, [[W, 128], [122 * W, 4], [1, W]])
    nc.gpsimd.dma_start(out=A1[:, 0:4, :], in_=src)
    src = AP(xin.tensor, (img * H + 488) * W, [[W, 128], [122 * W, 4], [1, W]])
    nc.gpsimd.dma_start(out=A1[:, 4:8, :], in_=src)
    nc.gpsimd.dma_start(out=A1[:, 8, :], in_=xin[img, 896:1024, :])

    B1 = apool.tile([128, 9, 1030], f16, tag="B1", bufs=AB_BUFS, name=f"B1_{img}")
    A2 = apool.tile([128, 5, 515], f16, tag="A2", bufs=3, name=f"A2_{img}")
    B2 = apool.tile([128, 5, 522], f16, tag="B2", bufs=3, name=f"B2_{img}")
    A3 = apool.tile([128, 3, 261], f16, tag="A3", bufs=3, name=f"A3_{img}")
    B3 = apool.tile([128, 3, 268], f16, tag="B3", bufs=3, name=f"B3_{img}")

    def p1a():
        s = _P1Sink(nc, 515, B1, f"p1a_{img}")
        _emit_pass(nc, psp, 1024, lambda si, m0: A1[:, si, m0:m0 + 128],
                   wsb[1024], s, rot)

    def p2a():
        cv1 = stpool.tile([128, 5, 515], f32, tag="st515", bufs=ST515_BUFS,
                          name=f"cv1_{img}")
        ch1 = stpool.tile([128, 5, 515], f32, tag="st515", bufs=ST515_BUFS,
                          name=f"ch1_{img}")
        cd1 = stpool.tile([128, 5, 515], f32, tag="st515", bufs=ST515_BUFS,
                          name=f"cd1_{img}")
        s = _P2Sink(nc, 1024, A2, cv1, ch1, cd1, None)
        s.tag = f"p2a_{img}"
        _emit_pass(nc, psp, 1024, lambda si, m0: B1[:, si, m0:m0 + 128],
                   wsb[1024], s, rot)
        _emit_det_dmas(nc, out, img, "cV1", cv1, [0, 122, 244, 366, 387], AP)
        _emit_det_dmas(nc, out, img, "cH1", ch1, [0, 128, 256, 384, 387], AP)
        _emit_det_dmas(nc, out, img, "cD1", cd1, [0, 128, 256, 384, 387], AP)

    def p1b():
        s = _P1Sink(nc, 261, B2, f"p1b_{img}")
        _emit_pass(nc, psp, 515, lambda si, m0: A2[:, si, m0:m0 + 128],
                   wsb[515], s, rot)

    def p2b():
        cv2 = stpool.tile([128, 3, 261], f32, tag="st261", bufs=6,
                          name=f"cv2_{img}")
        ch2 = stpool.tile([128, 3, 261], f32, tag="st261", bufs=6,
                          name=f"ch2_{img}")
        cd2 = stpool.tile([128, 3, 261], f32, tag="st261", bufs=6,
                          name=f"cd2_{img}")
        s = _P2Sink(nc, 515, A3, cv2, ch2, cd2, None)
        s.tag = f"p2b_{img}"
        _emit_pass(nc, psp, 515, lambda si, m0: B2[:, si, m0:m0 + 128],
                   wsb[515], s, rot)
        _emit_det_dmas(nc, out, img, "cV2", cv2, [0, 122, 133], AP)
        _emit_det_dmas(nc, out, img, "cH2", ch2, [0, 128, 133], AP)
        _emit_det_dmas(nc, out, img, "cD2", cd2, [0, 128, 133], AP)

    def p1c():
        s = _P1Sink(nc, 134, B3, f"p1c_{img}")
        _emit_pass(nc, psp, 261, lambda si, m0: A3[:, si, m0:m0 + 128],
                   wsb[261], s, rot)

    def p2c():
        ca3 = stpool.tile([128, 2, 134], f32, tag="st134", bufs=12,
                          name=f"ca3_{img}")
        cv3 = stpool.tile([128, 2, 134], f32, tag="st134", bufs=12,
                          name=f"cv3_{img}")
        ch3 = stpool.tile([128, 2, 134], f32, tag="st134", bufs=12,
                          name=f"ch3_{img}")
        cd3 = stpool.tile([128, 2, 134], f32, tag="st134", bufs=12,
                          name=f"cd3_{img}")
        s = _P2Sink(nc, 261, None, cv3, ch3, cd3, ca3)
        s.tag = f"p2c_{img}"
        _emit_pass(nc, psp, 261, lambda si, m0: B3[:, si, m0:m0 + 128],
                   wsb[261], s, rot)
        _emit_det_dmas(nc, out, img, "cA3", ca3, [0, 6], AP)
        _emit_det_dmas(nc, out, img, "cV3", cv3, [0, 6], AP)
        _emit_det_dmas(nc, out, img, "cH3", ch3, [0, 6], AP)
        _emit_det_dmas(nc, out, img, "cD3", cd3, [0, 6], AP)

    return [p1a, p2a, p1b, p2b, p1c, p2c]


# ----------------------------------------------------------------- runner
EXTRA_INPUTS = {f"w{N}": WEIGHTS[N] for N in LEVEL_NS}


def _get_built():
    global _BUILT
    if _BUILT is None:
        _BUILT = build_bass()
    return _BUILT


def kernel(x: np.ndarray) -> np.ndarray:
    from concourse import bass_utils

    x = np.ascontiguousarray(np.asarray(x), dtype=np.float32)
    assert x.shape == (B, C, H, W), x.shape
    nc = _get_built()

    imgs = x.reshape(B * C, H, W)
    in_maps = []
    for c in range(N_CORES):
        m = {"xin": imgs[c * IMGS_PER_CORE:(c + 1) * IMGS_PER_CORE]}
        m.update(EXTRA_INPUTS)
        in_maps.append(m)

    res = bass_utils.run_bass_kernel_spmd(nc, in_maps,
                                          core_ids=list(range(N_CORES)))
    outs = [res.results[c]["out"] for c in range(N_CORES)]
    flat = np.concatenate(outs, axis=0)  # [48, 1048576]
    return flat.reshape(B, C, 64, 128, 128)
